# revision 1
# baseline (speedup 1.0000x reference)
"""AttentiveItemToVec TRN2 kernel (8 NeuronCores, SPMD data-parallel over batch).

Math (per batch row b):
  v  = tvec[titems[b]]                 # [32, 128]
  u  = cvec[citems[b]]                 # [100, 128]
  tq = v @ At_w.T + At_b               # [32, 40]
  ck = u @ Ac_w.T + Ac_b               # [100, 40]
  cos[j, m] = <tq_j, ck_m> / (max(|tq_j|, eps) * max(|ck_m|, eps))
  cos[:, m] = -inf where (b, m) padded
  attn = softmax_m(cos)
  z = attn @ (u @ Bc_w.T + Bc_b) @ R_w.T + R_b
    = (E @ (u @ W2.T)) / rowsum(E) + b2        # E = exp(cos + mask), W2 = R_w@Bc_w,
                                               # b2 = R_w@Bc_b + R_b (uses sum(attn)=1)

Device strategy per core (128 batch rows):
  - host folds the A-projections into gather tables:
      cfull [V, 168] = [cvec | cvec@Ac_w.T + Ac_b],  tfull [V, 40] = tvec@At_w.T + At_b
  - 100 + 32 indirect-DMA gathers (128 rows each) -> token-major SBUF tiles
  - PE transposes -> E-major uT_all [128, 12800], ckT_all [40, 12800], tqT_all [40, 4096]
  - norms via ones-matmuls + DRAM-bounce relayouts; pad mask built with
    iota/is_equal one-hots + PE accumulation (no scatter)
  - per-b: dotT -> *invnc -> exp(+mask bias) -> ET; rowsum; Bu2 = uT_b.T @ W2T;
    z = ET.T @ Bu2 * invsum + b2; DMA out
"""
import sys

sys.path.insert(0, "/opt/trn_rl_repo")

import numpy as np

import concourse.bass as bass
import concourse.mybir as mybir
from concourse import bacc
from concourse.tile import TileContext
from concourse.bass_utils import run_bass_kernel_spmd

F32 = mybir.dt.float32
I32 = mybir.dt.int32
AF = mybir.ActivationFunctionType
OP = mybir.AluOpType

V, E, DA = 1_000_000, 128, 40
B, J, M = 1024, 32, 100
NCORES = 8
BL = B // NCORES          # 128 batch rows per core
CE = E + DA               # 168: folded context row
NT_C = BL * M // 128      # 100 c-gather tiles
NT_T = BL * J // 128      # 32 t-gather tiles
NPAD_CHUNKS = 34          # per-core pad-list capacity = 34*128 = 4352
NEG = -1e30
EPS = 1e-6

_trace = [False]          # test.py may flip this for profiling runs
_last_exec_ns = [None]


def _build_bass():
    nc = bacc.Bacc("TRN2", target_bir_lowering=False, debug=False,
                   num_devices=NCORES)

    cfull = nc.declare_dram_parameter("cfull", [V, CE], F32, isOutput=False)
    tfull = nc.declare_dram_parameter("tfull", [V, DA], F32, isOutput=False)
    cidx = nc.declare_dram_parameter("cidx", [128, NT_C], I32, isOutput=False)
    tidx = nc.declare_dram_parameter("tidx", [128, NT_T], I32, isOutput=False)
    padm = nc.declare_dram_parameter("padm", [128, NPAD_CHUNKS], I32, isOutput=False)
    padb = nc.declare_dram_parameter("padb", [128, NPAD_CHUNKS], I32, isOutput=False)
    w2t = nc.declare_dram_parameter("w2t", [E, E], F32, isOutput=False)
    identd = nc.declare_dram_parameter("identd", [128, 128], F32, isOutput=False)
    iotamd = nc.declare_dram_parameter("iotamd", [128, M], I32, isOutput=False)
    iotabd = nc.declare_dram_parameter("iotabd", [128, 128], I32, isOutput=False)
    b2bc = nc.declare_dram_parameter("b2bc", [J, E], F32, isOutput=False)
    zout = nc.declare_dram_parameter("zout", [BL, J, E], F32, isOutput=True)

    with TileContext(nc) as tc:
        with tc.tile_pool(name="const", bufs=1) as cp, \
             tc.tile_pool(name="big", bufs=1) as bigp, \
             tc.tile_pool(name="dram", bufs=1, space="DRAM") as dp:

            # ---------------- constants / small loads ----------------
            cidx_t = cp.tile([128, NT_C], I32)
            nc.sync.dma_start(out=cidx_t[:], in_=cidx[:, :])
            tidx_t = cp.tile([128, NT_T], I32)
            nc.sync.dma_start(out=tidx_t[:], in_=tidx[:, :])
            padm_t = cp.tile([128, NPAD_CHUNKS], I32)
            nc.sync.dma_start(out=padm_t[:], in_=padm[:, :])
            padb_t = cp.tile([128, NPAD_CHUNKS], I32)
            nc.sync.dma_start(out=padb_t[:], in_=padb[:, :])
            w2t_t = cp.tile([E, E], F32)
            nc.sync.dma_start(out=w2t_t[:], in_=w2t[:, :])
            b2bc_t = cp.tile([J, E], F32)
            nc.sync.dma_start(out=b2bc_t[:], in_=b2bc[:, :])

            ident = cp.tile([128, 128], F32)
            nc.sync.dma_start(out=ident[:], in_=identd[:, :])

            ones100 = cp.tile([M, 1], F32)
            nc.vector.memset(ones100[:], 1.0)
            ones40c = cp.tile([DA, 1], F32)
            nc.vector.memset(ones40c[:], 1.0)
            ones1x40 = cp.tile([1, DA], F32)
            nc.vector.memset(ones1x40[:], 1.0)

            # iotas for one-hot mask build
            iota_m = cp.tile([128, M], I32)
            nc.sync.dma_start(out=iota_m[:], in_=iotamd[:, :])
            iota_b = cp.tile([128, 128], I32)
            nc.sync.dma_start(out=iota_b[:], in_=iotabd[:, :])

            # persistent E-major arrays
            uT_all = bigp.tile([E, BL * M], F32)       # 50KB/part
            ckT_all = bigp.tile([DA, BL * M], F32)
            tqnT_all = bigp.tile([DA, BL * J], F32)
            negmT = bigp.tile([M, 128], F32)           # -1e30 * padcount, [m, b]
            invncT = bigp.tile([M, 128], F32)          # [m, b]
            ET_all = bigp.tile([M, BL * J], F32)       # exp(cos) per b, [m, 32b..]

            # DRAM bounce buffers
            ncsq_d = dp.tile([BL * M], F32, name="ncsq_d")
            ntsq_d = dp.tile([BL * J], F32, name="ntsq_d")
            invnt_d = dp.tile([BL * J], F32, name="invnt_d")

            # ---------------- pad mask (one-hot matmul accumulation) -------
            with tc.tile_pool(name="maskp", bufs=2) as mp, \
                 tc.tile_pool(name="maskps", bufs=1, space="PSUM") as mps:
                mask_ps = mps.tile([M, 128], F32, space="PSUM")
                for k in range(NPAD_CHUNKS):
                    oh_m = mp.tile([128, M], F32, tag="ohm", bufs=2)
                    oh_b = mp.tile([128, 128], F32, tag="ohb", bufs=2)
                    nc.vector.tensor_tensor(
                        out=oh_m[:], in0=iota_m[:],
                        in1=padm_t[:, k:k + 1].to_broadcast([128, M]),
                        op=OP.is_equal)
                    nc.vector.tensor_tensor(
                        out=oh_b[:], in0=iota_b[:],
                        in1=padb_t[:, k:k + 1].to_broadcast([128, 128]),
                        op=OP.is_equal)
                    nc.tensor.matmul(mask_ps[:], oh_m[:], oh_b[:],
                                     start=(k == 0), stop=(k == NPAD_CHUNKS - 1))
                nc.scalar.mul(negmT[:], mask_ps[:], NEG)

            # ---------------- t pipeline: gathers -> tqT_all -> tqnT_all ----
            with tc.tile_pool(name="traw", bufs=8) as trp, \
                 tc.tile_pool(name="tps", bufs=2, space="PSUM") as tps:
                for s in range(NT_T):
                    t_raw = trp.tile([128, DA], F32, tag="traw", bufs=8)
                    nc.gpsimd.indirect_dma_start(
                        out=t_raw[:], out_offset=None, in_=tfull[:, :],
                        in_offset=bass.IndirectOffsetOnAxis(
                            ap=tidx_t[:, s:s + 1], axis=0))
                    tp = tps.tile([DA, 128], F32, space="PSUM", tag="tp", bufs=2)
                    nc.tensor.transpose(tp[:], t_raw[:], ident[:])
                    # copy into tqT staging (reuse tqnT_all buffer pre-normalization)
                    if s % 2 == 0:
                        nc.scalar.copy(tqnT_all[:, s * 128:(s + 1) * 128], tp[:])
                    else:
                        nc.vector.tensor_copy(tqnT_all[:, s * 128:(s + 1) * 128], tp[:])

                # ntsq chunks: [1, 512] = sum_da tq^2, via ones-matmul
                with tc.tile_pool(name="tsq", bufs=2) as tsqp, \
                     tc.tile_pool(name="tnps", bufs=2, space="PSUM") as tnps:
                    for k in range(BL * J // 512):
                        sl = slice(k * 512, (k + 1) * 512)
                        sq = tsqp.tile([DA, 512], F32, tag="tsq", bufs=2)
                        nc.scalar.square(sq[:], tqnT_all[:, sl])
                        nps = tnps.tile([1, 512], F32, space="PSUM", tag="nps", bufs=2)
                        nc.tensor.matmul(nps[:], ones40c[:], sq[:])
                        row = tsqp.tile([1, 512], F32, tag="trow", bufs=2)
                        nc.vector.tensor_copy(row[:], nps[:])
                        nc.sync.dma_start(out=ntsq_d[sl][None, :], in_=row[:, :])
                # bounce: [4096] -> [128, 32], chain, -> [4096] -> bcast -> mult
                ntsq_bj = cp.tile([128, J], F32)
                nc.sync.dma_start(
                    out=ntsq_bj[:],
                    in_=ntsq_d[:].rearrange("(b j) -> b j", b=128))
                nc.scalar.sqrt(ntsq_bj[:], ntsq_bj[:])
                nc.vector.tensor_scalar_max(ntsq_bj[:], ntsq_bj[:], EPS)
                nc.vector.reciprocal(ntsq_bj[:], ntsq_bj[:])
                nc.sync.dma_start(
                    out=invnt_d[:].rearrange("(b j) -> b j", b=128), in_=ntsq_bj[:])
                invnt_row = cp.tile([1, BL * J], F32)
                nc.sync.dma_start(out=invnt_row[:, :], in_=invnt_d[:][None, :])
                with tc.tile_pool(name="tbc", bufs=2) as tbcp, \
                     tc.tile_pool(name="tbps", bufs=2, space="PSUM") as tbps:
                    for k in range(BL * J // 512):
                        sl = slice(k * 512, (k + 1) * 512)
                        bps = tbps.tile([DA, 512], F32, space="PSUM", tag="bps", bufs=2)
                        nc.tensor.matmul(bps[:], ones1x40[:], invnt_row[:, sl])
                        bsb = tbcp.tile([DA, 512], F32, tag="bsb", bufs=2)
                        nc.scalar.copy(bsb[:], bps[:])
                        nc.vector.tensor_tensor(out=tqnT_all[:, sl],
                                                in0=tqnT_all[:, sl], in1=bsb[:],
                                                op=OP.mult)

            # ---------------- main: c gathers + transposes + per-b passes ---
            from contextlib import ExitStack
            _main_ctx = ExitStack()
            craw_p = _main_ctx.enter_context(tc.tile_pool(name="craw", bufs=16))
            cps_p = _main_ctx.enter_context(tc.tile_pool(name="cps", bufs=4, space="PSUM"))
            work_p = _main_ctx.enter_context(tc.tile_pool(name="work", bufs=4))
            mainps_p = _main_ctx.enter_context(tc.tile_pool(name="mainps", bufs=4, space="PSUM"))

            NCSQ_CH = 512
            n_ncsq = BL * M // NCSQ_CH      # 25 chunks
            next_ncsq = 0
            next_inv = 0                     # invnc chunks of 8 b's
            next_b1 = 0                      # pass-1 b
            next_b2 = 0                      # pass-2 b

            def emit_ncsq(k):
                sl = slice(k * NCSQ_CH, (k + 1) * NCSQ_CH)
                sq = work_p.tile([DA, NCSQ_CH], F32, tag="csq", bufs=2)
                nc.scalar.square(sq[:], ckT_all[:, sl])
                nps = mainps_p.tile([1, NCSQ_CH], F32, space="PSUM", tag="smallp", bufs=3)
                nc.tensor.matmul(nps[:], ones40c[:], sq[:])
                row = work_p.tile([1, NCSQ_CH], F32, tag="crow", bufs=2)
                nc.vector.tensor_copy(row[:], nps[:])
                nc.sync.dma_start(out=ncsq_d[sl][None, :], in_=row[:, :])

            def emit_invnc(g):
                # 8 b's: tokens [800g, 800g+800) -> [8, 100] -> chain -> T -> [100, 8]
                sl = slice(g * 8 * M, (g + 1) * 8 * M)
                t8 = work_p.tile([8, M], F32, tag="i8", bufs=2)
                nc.sync.dma_start(out=t8[:],
                                  in_=ncsq_d[sl].rearrange("(b m) -> b m", b=8))
                nc.scalar.sqrt(t8[:], t8[:])
                nc.vector.tensor_scalar_max(t8[:], t8[:], EPS)
                nc.vector.reciprocal(t8[:], t8[:])
                ip = mainps_p.tile([M, 8], F32, space="PSUM", tag="smallp", bufs=3)
                nc.tensor.transpose(ip[:], t8[:], ident[:8, :8])
                nc.scalar.copy(invncT[:, g * 8:(g + 1) * 8], ip[:])

            def emit_pass1(b):
                slm = slice(b * M, (b + 1) * M)
                slj = slice(b * J, (b + 1) * J)
                dps = mainps_p.tile([M, J], F32, space="PSUM", tag="smallp", bufs=3)
                nc.tensor.matmul(dps[:], ckT_all[:, slm], tqnT_all[:, slj])
                cosn = work_p.tile([M, J], F32, tag="cosn", bufs=3)
                nc.vector.tensor_scalar_mul(cosn[:], dps[:],
                                            invncT[:, b:b + 1])
                nc.scalar.activation(ET_all[:, slj], cosn[:], AF.Exp,
                                     bias=negmT[:, b:b + 1], scale=1.0)
                rs = mainps_p.tile([J, 1], F32, space="PSUM", tag="smallp", bufs=3)
                nc.tensor.matmul(rs[:], ET_all[:, slj], ones100[:])
                inv = work_p.tile([J, 1], F32, tag="inv", bufs=3, name=f"inv_{b}")
                nc.vector.reciprocal(inv[:], rs[:])
                return inv

            inv_tiles = {}

            def emit_pass2(b):
                slm = slice(b * M, (b + 1) * M)
                slj = slice(b * J, (b + 1) * J)
                bps = mainps_p.tile([M, E], F32, space="PSUM", tag="bu2", bufs=1)
                nc.tensor.matmul(bps[:], uT_all[:, slm], w2t_t[:])
                bsb = work_p.tile([M, E], F32, tag="bu2s", bufs=2)
                if b % 2 == 0:
                    nc.scalar.copy(bsb[:], bps[:])
                else:
                    nc.vector.tensor_copy(bsb[:], bps[:])
                zps = mainps_p.tile([J, E], F32, space="PSUM", tag="z", bufs=1)
                nc.tensor.matmul(zps[:], ET_all[:, slj], bsb[:])
                zsb = work_p.tile([J, E], F32, tag="zsb", bufs=3)
                nc.vector.tensor_scalar_mul(zsb[:], zps[:], inv_tiles[b][:, :1])
                nc.vector.tensor_tensor(out=zsb[:], in0=zsb[:], in1=b2bc_t[:],
                                        op=OP.add)
                nc.sync.dma_start(out=zout[b], in_=zsb[:])

            for jt in range(NT_C):
                c_raw = craw_p.tile([128, CE], F32, tag="craw", bufs=16)
                nc.gpsimd.indirect_dma_start(
                    out=c_raw[:], out_offset=None, in_=cfull[:, :],
                    in_offset=bass.IndirectOffsetOnAxis(
                        ap=cidx_t[:, jt:jt + 1], axis=0))
                up = cps_p.tile([128, 128], F32, space="PSUM", tag="up", bufs=2)
                nc.tensor.transpose(up[:], c_raw[:, 0:E], ident[:])
                kp = cps_p.tile([DA, 128], F32, space="PSUM", tag="kp", bufs=1)
                nc.tensor.transpose(kp[:], c_raw[:, E:CE], ident[:])
                csl = slice(jt * 128, (jt + 1) * 128)
                if jt % 2 == 0:
                    nc.scalar.copy(uT_all[:, csl], up[:])
                    nc.vector.tensor_copy(ckT_all[:, csl], kp[:])
                else:
                    nc.vector.tensor_copy(uT_all[:, csl], up[:])
                    nc.scalar.copy(ckT_all[:, csl], kp[:])

                tok_done = (jt + 1) * 128
                while next_ncsq < n_ncsq and (next_ncsq + 1) * NCSQ_CH <= tok_done:
                    emit_ncsq(next_ncsq)
                    next_ncsq += 1
                while next_inv < 16 and (next_inv + 1) * 8 * M <= next_ncsq * NCSQ_CH:
                    emit_invnc(next_inv)
                    next_inv += 1
                while next_b1 < BL and (next_b1 + 1) * M <= tok_done \
                        and (next_b1 // 8) < next_inv:
                    inv_tiles[next_b1] = emit_pass1(next_b1)
                    next_b1 += 1
                while next_b2 < next_b1:
                    emit_pass2(next_b2)
                    next_b2 += 1

            while next_ncsq < n_ncsq:
                emit_ncsq(next_ncsq)
                next_ncsq += 1
            while next_inv < 16:
                emit_invnc(next_inv)
                next_inv += 1
            while next_b1 < BL:
                inv_tiles[next_b1] = emit_pass1(next_b1)
                next_b1 += 1
            while next_b2 < BL:
                emit_pass2(next_b2)
                next_b2 += 1

            _main_ctx.close()

    nc.finalize()
    return nc


_nc_cache = [None]


def kernel(batch_titems, batch_citems, pad_rows, pad_cols, tvec, cvec,
           Ac_w, Ac_b, At_w, At_b, Bc_w, Bc_b, R_w, R_b):
    batch_titems = np.asarray(batch_titems).astype(np.int32)
    batch_citems = np.asarray(batch_citems).astype(np.int32)
    pad_rows = np.asarray(pad_rows).astype(np.int64)
    pad_cols = np.asarray(pad_cols).astype(np.int64)
    tvec = np.asarray(tvec, dtype=np.float32)
    cvec = np.asarray(cvec, dtype=np.float32)
    Ac_w = np.asarray(Ac_w, dtype=np.float32)
    Ac_b = np.asarray(Ac_b, dtype=np.float32)
    At_w = np.asarray(At_w, dtype=np.float32)
    At_b = np.asarray(At_b, dtype=np.float32)
    Bc_w = np.asarray(Bc_w, dtype=np.float32)
    Bc_b = np.asarray(Bc_b, dtype=np.float32)
    R_w = np.asarray(R_w, dtype=np.float32)
    R_b = np.asarray(R_b, dtype=np.float32)

    # ---- host weight folding ----
    cfull = np.empty((V, CE), dtype=np.float32)
    cfull[:, :E] = cvec
    cfull[:, E:] = cvec @ Ac_w.T + Ac_b
    tfull = (tvec @ At_w.T + At_b).astype(np.float32)
    W2 = R_w @ Bc_w                                   # [E, E]
    w2t = np.ascontiguousarray(W2.T, dtype=np.float32)
    b2 = R_w @ Bc_b + R_b                             # [E]
    b2bc = np.broadcast_to(b2, (J, E)).copy()

    _ident_np = np.eye(128, dtype=np.float32)
    _iotam_np = np.broadcast_to(np.arange(M, dtype=np.int32), (128, M)).copy()
    _iotab_np = np.broadcast_to(np.arange(128, dtype=np.int32), (128, 128)).copy()
    in_maps = []
    for c in range(NCORES):
        b0 = c * BL
        cit = batch_citems[b0:b0 + BL].ravel()        # [12800]
        tit = batch_titems[b0:b0 + BL].ravel()        # [4096]
        cidx = np.ascontiguousarray(cit.reshape(NT_C, 128).T.astype(np.int32))
        tidx = np.ascontiguousarray(tit.reshape(NT_T, 128).T.astype(np.int32))
        sel = (pad_rows >= b0) & (pad_rows < b0 + BL)
        pm = pad_cols[sel].astype(np.int32)
        pb = (pad_rows[sel] - b0).astype(np.int32)
        cap = NPAD_CHUNKS * 128
        if pm.size > cap:
            raise RuntimeError(f"pad capacity exceeded: {pm.size} > {cap}")
        padm = np.full(cap, 999, dtype=np.int32)
        padb = np.zeros(cap, dtype=np.int32)
        padm[:pm.size] = pm
        padb[:pb.size] = pb
        in_maps.append({
            "cfull": cfull, "tfull": tfull,
            "cidx": cidx, "tidx": tidx,
            "padm": np.ascontiguousarray(padm.reshape(NPAD_CHUNKS, 128).T),
            "padb": np.ascontiguousarray(padb.reshape(NPAD_CHUNKS, 128).T),
            "w2t": w2t, "b2bc": b2bc,
            "identd": _ident_np, "iotamd": _iotam_np, "iotabd": _iotab_np,
        })

    if _nc_cache[0] is None:
        _nc_cache[0] = _build_bass()
    nc = _nc_cache[0]

    res = run_bass_kernel_spmd(nc, in_maps, list(range(NCORES)),
                               trace=_trace[0])
    _last_exec_ns[0] = res.exec_time_ns
    z = np.concatenate([r["zout"] for r in res.results], axis=0)
    return z.astype(np.float32)



# revision 2
# speedup vs baseline: 1.5384x; 1.5384x over previous
"""AttentiveItemToVec TRN2 kernel (8 NeuronCores, SPMD data-parallel over batch).

Math per batch row b (J=32 targets, M=100 contexts, E=128, DA=40):
  cos[j,m] = <tqn_j, ckn_m> with tqn/ckn the A-projected, per-VOCAB-normalized
             embeddings (norms are pure functions of the vocab row -> host).
  attn = softmax_m(cos + mask);  z = (attn @ u) @ W2^T + b2
             (W2 = R_w@Bc_w, b2 = R_w@Bc_b + R_b, using sum(attn)=1)

Device strategy per core (128 batch rows = 12800 c-tokens, 4096 t-tokens):
  - gather tables (host-precomputed, bf16):
      cfull [1M, 168] = [cvec | ckn],  tfull [1M, 41] = [tqn | 1.0]
  - 100 + 32 indirect row-gathers (128 rows each; the [P,1]-offset form is
    the only working indirect primitive, ~1us SWDGE each -> the wall floor)
  - E-major relayouts via DRAM bounce + XBAR DMA-transpose (no PE transposes)
  - pad mask folded into the dot matmul as contraction row 40 (lhsT row 40 =
    0/-1e30 per token, rhs row 40 = 1.0 from the table)
  - per b: dot -> exp -> (batched) rowsum/alpha; endgame: one W2 matmul per
    512 tokens, softmax denominators applied per-column via K=1 replicate
    matmul, output transposed back token-major with XBAR, stored bf16.
"""
import sys

sys.path.insert(0, "/opt/trn_rl_repo")

import numpy as np
import ml_dtypes

import concourse.bass as bass
import concourse.mybir as mybir
from concourse import bacc
from concourse.tile import TileContext
from concourse.bass_utils import run_bass_kernel_spmd

F32 = mybir.dt.float32
BF16 = mybir.dt.bfloat16
I32 = mybir.dt.int32
AF = mybir.ActivationFunctionType
OP = mybir.AluOpType

V, E, DA = 1_000_000, 128, 40
B, J, M = 1024, 32, 100
NCORES = 8
BL = B // NCORES            # 128 batch rows per core
CE = E + DA                 # 168 fused c row: [u(128) | ckn(40)]
TW = DA + 1                 # 41 t row: [tqn(40) | 1.0]
NTC = BL * M // 128         # 100 c-gather tiles
NTT = BL * J // 128         # 32 t-gather tiles
NTOK = BL * M               # 12800 c tokens
TTOK = BL * J               # 4096 t tokens
NCH = 4                     # chunks of 32 b's
BPC = BL // NCH             # 32 b per chunk
TPC = NTC // NCH            # 25 c tiles per chunk
SUB = 5                     # c tiles per bounce sub-write
NEG = -1e30

_trace = [False]
_last_exec_ns = [None]


def _build_bass():
    nc = bacc.Bacc("TRN2", target_bir_lowering=False, debug=False,
                   num_devices=NCORES)

    cfull = nc.declare_dram_parameter("cfull", [V, CE], BF16, isOutput=False)
    tfull = nc.declare_dram_parameter("tfull", [V, TW], BF16, isOutput=False)
    cidx = nc.declare_dram_parameter("cidx", [128, NTC], I32, isOutput=False)
    tidx = nc.declare_dram_parameter("tidx", [128, NTT], I32, isOutput=False)
    negm = nc.declare_dram_parameter("negm", [1, NTOK], BF16, isOutput=False)
    w2t = nc.declare_dram_parameter("w2t", [E, E], BF16, isOutput=False)
    b2c = nc.declare_dram_parameter("b2c", [E, 1], F32, isOutput=False)
    zout = nc.declare_dram_parameter("zout", [TTOK, E], BF16, isOutput=True)

    with TileContext(nc) as tc:
        with tc.tile_pool(name="const", bufs=1) as cp, \
             tc.tile_pool(name="big", bufs=1) as bigp, \
             tc.tile_pool(name="dram", bufs=1, space="DRAM") as dp, \
             tc.tile_pool(name="cg", bufs=3) as cgp, \
             tc.tile_pool(name="tg", bufs=2) as tgp, \
             tc.tile_pool(name="wrk", bufs=2) as wp, \
             tc.tile_pool(name="zt", bufs=2) as zp, \
             tc.tile_pool(name="dotps", bufs=2, space="PSUM") as dotp, \
             tc.tile_pool(name="alps", bufs=2, space="PSUM") as alp, \
             tc.tile_pool(name="rsps", bufs=2, space="PSUM") as rsp, \
             tc.tile_pool(name="rbps", bufs=1, space="PSUM") as rbp, \
             tc.tile_pool(name="zps", bufs=1, space="PSUM") as zpp:

            # ---- constants ----
            cidx_t = cp.tile([128, NTC], I32)
            nc.sync.dma_start(out=cidx_t[:], in_=cidx[:, :])
            tidx_t = cp.tile([128, NTT], I32)
            nc.sync.dma_start(out=tidx_t[:], in_=tidx[:, :])
            w2t_t = cp.tile([E, E], BF16)
            nc.sync.dma_start(out=w2t_t[:], in_=w2t[:, :])
            b2_t = cp.tile([E, 1], F32)
            nc.sync.dma_start(out=b2_t[:], in_=b2c[:, :])
            ones100 = cp.tile([M, 1], BF16)
            nc.vector.memset(ones100[:], 1.0)
            ones1f = cp.tile([1, 128], F32)
            nc.vector.memset(ones1f[:], 1.0)

            # ---- persistent arrays ----
            cknT = bigp.tile([128, NTOK], BF16)      # rows 0:40 ckn, 40 mask
            tqnT = bigp.tile([128, TTOK], BF16)      # rows 0:40 tqn, 40 ones
            u_all = bigp.tile([M, BL, E], BF16)      # m-part, b, e
            ET_all = bigp.tile([M, TTOK], BF16)      # exp(cos+mask), m-part
            alphaT = bigp.tile([E, TTOK], BF16)      # E-major alpha (unnorm)
            zfinT = bigp.tile([E, TTOK], BF16)       # E-major final z

            ub_d = dp.tile([NTOK, E], BF16, name="ub_d")
            ckb_d = dp.tile([NTOK, 128], BF16, name="ckb_d")
            tqb_d = dp.tile([TTOK, 128], BF16, name="tqb_d")

            rs_tiles = {}

            def emit_group(g):
                """4 b's: dot matmuls -> exp -> rowsum -> alpha."""
                dps = dotp.tile([M, 128], F32, space="PSUM", tag="dot")
                for r in range(4):
                    b = 4 * g + r
                    nc.tensor.matmul(
                        dps[:, 32 * r:32 * (r + 1)],
                        cknT[0:TW, M * b:M * (b + 1)],
                        tqnT[0:TW, J * b:J * (b + 1)],
                        start=True, stop=True)
                sl = slice(128 * g, 128 * (g + 1))
                nc.scalar.activation(ET_all[:, sl], dps[:], AF.Exp)
                s = g // 4
                if s not in rs_tiles:
                    rs_tiles[s] = rsp.tile([1, 512], F32, space="PSUM",
                                           tag="rs", name=f"rs_{s}")
                nc.tensor.matmul(
                    rs_tiles[s][:, 128 * (g % 4):128 * (g % 4 + 1)],
                    ones100[:], ET_all[:, sl], start=True, stop=True)
                aps = alp.tile([E, 128], F32, space="PSUM", tag="alpha")
                for r in range(4):
                    b = 4 * g + r
                    nc.tensor.matmul(
                        aps[:, 32 * r:32 * (r + 1)],
                        u_all[:, b, :], ET_all[:, J * b:J * (b + 1)],
                        start=True, stop=True)
                nc.vector.tensor_copy(alphaT[:, sl], aps[:])

            def emit_s512(s):
                """512 tokens: denominators + W2 + bias into zfinT."""
                sl = slice(512 * s, 512 * (s + 1))
                inv_row = wp.tile([1, 512], F32, tag="inv")
                nc.vector.reciprocal(inv_row[:], rs_tiles[s][:])
                rsb = rbp.tile([E, 512], F32, space="PSUM", tag="rsb")
                nc.tensor.matmul(rsb[:], ones1f[:], inv_row[:],
                                 start=True, stop=True)
                rsb_s = wp.tile([E, 512], F32, tag="rsbs")
                nc.vector.tensor_copy(rsb_s[:], rsb[:])
                zps = zpp.tile([E, 512], F32, space="PSUM", tag="z")
                nc.tensor.matmul(zps[:], w2t_t[:], alphaT[:, sl],
                                 start=True, stop=True)
                zt1 = wp.tile([E, 512], F32, tag="zt1")
                nc.vector.tensor_tensor(out=zt1[:], in0=zps[:], in1=rsb_s[:],
                                        op=OP.mult)
                nc.vector.tensor_tensor(out=zfinT[:, sl], in0=zt1[:],
                                        in1=b2_t[:].to_broadcast([E, 512]),
                                        op=OP.add)

            for q in range(NCH):
                # -- c gathers + bounce + transpose, SUB tiles at a time --
                for sgrp in range(TPC // SUB):
                    c_tile = cgp.tile([128, SUB, CE], BF16, tag="cg")
                    for i in range(SUB):
                        jt = TPC * q + SUB * sgrp + i
                        nc.gpsimd.indirect_dma_start(
                            out=c_tile[:, i, :], out_offset=None,
                            in_=cfull[:, :],
                            in_offset=bass.IndirectOffsetOnAxis(
                                ap=cidx_t[:, jt:jt + 1], axis=0))
                    r0 = 128 * (TPC * q + SUB * sgrp)
                    nrow = 128 * SUB
                    nc.sync.dma_start(
                        out=ub_d[r0:r0 + nrow, :].rearrange(
                            "(i p) e -> p i e", p=128),
                        in_=c_tile[:, :, 0:E])
                    nc.sync.dma_start(
                        out=ckb_d[r0:r0 + nrow, 0:DA].rearrange(
                            "(i p) e -> p i e", p=128),
                        in_=c_tile[:, :, E:CE])
                    nc.sync.dma_start(out=cknT[:, r0:r0 + nrow],
                                      in_=ckb_d[r0:r0 + nrow, :],
                                      transpose=True)
                # mask row for this chunk (after the transposes above)
                c0 = NTOK // NCH * q
                nc.sync.dma_start(
                    out=cknT[DA:DA + 1, c0:c0 + NTOK // NCH],
                    in_=negm[:, c0:c0 + NTOK // NCH])
                # -- t gathers + bounce + transpose (8 tiles per chunk) --
                for th in range(2):
                    t_tile = tgp.tile([128, 4, TW], BF16, tag="tg")
                    for i in range(4):
                        st = 8 * q + 4 * th + i
                        nc.gpsimd.indirect_dma_start(
                            out=t_tile[:, i, :], out_offset=None,
                            in_=tfull[:, :],
                            in_offset=bass.IndirectOffsetOnAxis(
                                ap=tidx_t[:, st:st + 1], axis=0))
                    t0 = 128 * (8 * q + 4 * th)
                    nc.sync.dma_start(
                        out=tqb_d[t0:t0 + 512, 0:TW].rearrange(
                            "(i p) e -> p i e", p=128),
                        in_=t_tile[:])
                    nc.sync.dma_start(out=tqnT[:, t0:t0 + 512],
                                      in_=tqb_d[t0:t0 + 512, :],
                                      transpose=True)
                # -- u readback (m-part, per-b layout) --
                nc.sync.dma_start(
                    out=u_all[:, BPC * q:BPC * (q + 1), :],
                    in_=ub_d[NTOK // NCH * q:NTOK // NCH * (q + 1), :]
                        .rearrange("(b m) e -> m b e", b=BPC))
                # -- compute: 8 groups of 4 b's, then per-512 endgame --
                for gg in range(8):
                    g = 8 * q + gg
                    emit_group(g)
                    if g % 4 == 3:
                        emit_s512(g // 4)
                # -- output transpose + store for this chunk --
                ztok = zp.tile([128, 8, E], BF16, tag="ztok")
                nc.sync.dma_start(out=ztok[:],
                                  in_=zfinT[:, 1024 * q:1024 * (q + 1)],
                                  transpose=True)
                nc.sync.dma_start(
                    out=zout[1024 * q:1024 * (q + 1), :].rearrange(
                        "(i p) e -> p i e", p=128),
                    in_=ztok[:])

    nc.finalize()
    return nc


_nc_cache = [None]


def kernel(batch_titems, batch_citems, pad_rows, pad_cols, tvec, cvec,
           Ac_w, Ac_b, At_w, At_b, Bc_w, Bc_b, R_w, R_b):
    batch_titems = np.asarray(batch_titems).astype(np.int32)
    batch_citems = np.asarray(batch_citems).astype(np.int32)
    pad_rows = np.asarray(pad_rows).astype(np.int64)
    pad_cols = np.asarray(pad_cols).astype(np.int64)
    tvec = np.asarray(tvec, dtype=np.float32)
    cvec = np.asarray(cvec, dtype=np.float32)
    Ac_w = np.asarray(Ac_w, dtype=np.float32)
    Ac_b = np.asarray(Ac_b, dtype=np.float32)
    At_w = np.asarray(At_w, dtype=np.float32)
    At_b = np.asarray(At_b, dtype=np.float32)
    Bc_w = np.asarray(Bc_w, dtype=np.float32)
    Bc_b = np.asarray(Bc_b, dtype=np.float32)
    R_w = np.asarray(R_w, dtype=np.float32)
    R_b = np.asarray(R_b, dtype=np.float32)

    # ---- host folding: normalized projection tables, fused W2/b2 ----
    ck = cvec @ Ac_w.T + Ac_b                        # [V, 40]
    nck = np.maximum(np.linalg.norm(ck, axis=1, keepdims=True), 1e-6)
    cfull = np.empty((V, CE), dtype=ml_dtypes.bfloat16)
    cfull[:, :E] = cvec.astype(ml_dtypes.bfloat16)
    cfull[:, E:] = (ck / nck).astype(ml_dtypes.bfloat16)
    tq = tvec @ At_w.T + At_b                        # [V, 40]
    ntq = np.maximum(np.linalg.norm(tq, axis=1, keepdims=True), 1e-6)
    tfull = np.ones((V, TW), dtype=ml_dtypes.bfloat16)
    tfull[:, :DA] = (tq / ntq).astype(ml_dtypes.bfloat16)
    W2 = R_w @ Bc_w
    w2t = np.ascontiguousarray(W2.T).astype(ml_dtypes.bfloat16)
    b2 = (R_w @ Bc_b + R_b).astype(np.float32).reshape(E, 1)

    in_maps = []
    for c in range(NCORES):
        b0 = c * BL
        cit = batch_citems[b0:b0 + BL].ravel()       # [12800] b-major
        tit = batch_titems[b0:b0 + BL].ravel()       # [4096]
        cidx = np.ascontiguousarray(cit.reshape(NTC, 128).T.astype(np.int32))
        tidx = np.ascontiguousarray(tit.reshape(NTT, 128).T.astype(np.int32))
        sel = (pad_rows >= b0) & (pad_rows < b0 + BL)
        negm = np.zeros((1, NTOK), dtype=ml_dtypes.bfloat16)
        flat = (pad_rows[sel] - b0) * M + pad_cols[sel]
        negm[0, flat] = NEG
        in_maps.append({
            "cfull": cfull, "tfull": tfull,
            "cidx": cidx, "tidx": tidx, "negm": negm,
            "w2t": w2t, "b2c": b2,
        })

    if _nc_cache[0] is None:
        _nc_cache[0] = _build_bass()
    nc = _nc_cache[0]

    res = run_bass_kernel_spmd(nc, in_maps, list(range(NCORES)),
                               trace=_trace[0])
    _last_exec_ns[0] = res.exec_time_ns
    z = np.stack([r["zout"].astype(np.float32).reshape(BL, J, E)
                  for r in res.results], axis=0)
    return z.reshape(B, J, E)


# revision 3
# speedup vs baseline: 1.6399x; 1.0659x over previous
"""AttentiveItemToVec TRN2 kernel (8 NeuronCores, SPMD data-parallel over batch).

Math per batch row b (J=32 targets, M=100 contexts, E=128, DA=40):
  cos[j,m] = <tqn_j, ckn_m> with tqn/ckn the A-projected, per-VOCAB-normalized
             embeddings (norms are pure functions of the vocab row -> host).
  attn = softmax_m(cos + mask);  z = (attn @ u) @ W2^T + b2
             (W2 = R_w@Bc_w, b2 = R_w@Bc_b + R_b, using sum(attn)=1)

Device strategy per core (128 batch rows = 12800 c-tokens, 4096 t-tokens):
  - gather tables (host-precomputed, bf16):
      cfull [1M, 168] = [cvec | ckn],  tfull [1M, 41] = [tqn | 1.0]
  - 100 + 32 indirect row-gathers (128 rows each; the [P,1]-offset form is
    the only working indirect primitive, ~1.4us/instr on the gpsimd queue ->
    the wall-clock floor; everything else is hidden under the gather stream)
  - E-major relayouts via DRAM bounce + XBAR DMA-transpose (no PE transposes)
  - pad mask folded into the dot matmul as contraction row 40 (lhsT row 40 =
    0/-1e30 per token, rhs row 40 = 1.0 from the table)
  - endgame: z^T = W2 @ alphaT (+ b2 x rowsum rank-1 accumulated into the same
    PSUM); divide by rowsum AFTER the token-major XBAR transpose where the
    denominator is a per-partition-parallel [128, 8] reciprocal
"""
import sys

sys.path.insert(0, "/opt/trn_rl_repo")

import numpy as np
import ml_dtypes

import concourse.bass as bass
import concourse.mybir as mybir
from concourse import bacc
from concourse.tile import TileContext
from concourse.bass_utils import run_bass_kernel_spmd

F32 = mybir.dt.float32
BF16 = mybir.dt.bfloat16
I32 = mybir.dt.int32
AF = mybir.ActivationFunctionType
OP = mybir.AluOpType

V, E, DA = 1_000_000, 128, 40
B, J, M = 1024, 32, 100
NCORES = 8
BL = B // NCORES            # 128 batch rows per core
CE = E + DA                 # 168 fused c row: [u(128) | ckn(40)]
TW = DA + 1                 # 41 t row: [tqn(40) | 1.0]
NTC = BL * M // 128         # 100 c-gather tiles
NTT = BL * J // 128         # 32 t-gather tiles
NTOK = BL * M               # 12800 c tokens
TTOK = BL * J               # 4096 t tokens
NCH = 4                     # chunks of 32 b's
BPC = BL // NCH             # 32 b per chunk
TPC = NTC // NCH            # 25 c tiles per chunk
SUB = 5                     # c tiles per bounce sub-write
NEG = -1e30

_trace = [False]
_last_exec_ns = [None]


def _build_bass():
    nc = bacc.Bacc("TRN2", target_bir_lowering=False, debug=False,
                   num_devices=NCORES)

    cfull = nc.declare_dram_parameter("cfull", [V, CE], BF16, isOutput=False)
    tfull = nc.declare_dram_parameter("tfull", [V, TW], BF16, isOutput=False)
    cidx = nc.declare_dram_parameter("cidx", [128, NTC], I32, isOutput=False)
    tidx = nc.declare_dram_parameter("tidx", [128, NTT], I32, isOutput=False)
    negm = nc.declare_dram_parameter("negm", [1, NTOK], BF16, isOutput=False)
    w2t = nc.declare_dram_parameter("w2t", [E, E], BF16, isOutput=False)
    b2r = nc.declare_dram_parameter("b2r", [1, E], BF16, isOutput=False)
    zout = nc.declare_dram_parameter("zout", [TTOK, E], BF16, isOutput=True)

    with TileContext(nc) as tc:
        with tc.tile_pool(name="const", bufs=1) as cp, \
             tc.tile_pool(name="big", bufs=1) as bigp, \
             tc.tile_pool(name="dram", bufs=1, space="DRAM") as dp, \
             tc.tile_pool(name="cg", bufs=8) as cgp, \
             tc.tile_pool(name="tg", bufs=4) as tgp, \
             tc.tile_pool(name="wrk", bufs=2) as wp, \
             tc.tile_pool(name="zt", bufs=2) as zp, \
             tc.tile_pool(name="dotps", bufs=2, space="PSUM") as dotp, \
             tc.tile_pool(name="alps", bufs=2, space="PSUM") as alp, \
             tc.tile_pool(name="rsps", bufs=2, space="PSUM") as rsp, \
             tc.tile_pool(name="rs2ps", bufs=1, space="PSUM") as rs2p, \
             tc.tile_pool(name="zps", bufs=1, space="PSUM") as zpp:

            # ---- constants ----
            cidx_t = cp.tile([128, NTC], I32)
            nc.sync.dma_start(out=cidx_t[:], in_=cidx[:, :])
            tidx_t = cp.tile([128, NTT], I32)
            nc.sync.dma_start(out=tidx_t[:], in_=tidx[:, :])
            w2t_t = cp.tile([E, E], BF16)
            nc.sync.dma_start(out=w2t_t[:], in_=w2t[:, :])
            b2_t = cp.tile([1, E], BF16)
            nc.sync.dma_start(out=b2_t[:], in_=b2r[:, :])
            ones100 = cp.tile([M, 1], BF16)
            nc.vector.memset(ones100[:], 1.0)

            # ---- persistent arrays ----
            cknT = bigp.tile([128, NTOK], BF16)      # rows 0:40 ckn, 40 mask
            tqnT = bigp.tile([128, TTOK], BF16)      # rows 0:40 tqn, 40 ones
            u_all = bigp.tile([M, BL, E], BF16)      # m-part, b, e
            ET_all = bigp.tile([M, TTOK], BF16)      # exp(cos+mask), m-part
            alphaT = bigp.tile([E, TTOK], BF16)      # E-major alpha (unnorm)
            zfinT = bigp.tile([E, TTOK], BF16)       # E-major z*rowsum
            inv_all = bigp.tile([128, NTT], F32)     # 1/rowsum, token-major

            ub_d = dp.tile([NTOK, E], BF16, name="ub_d")
            ckb_d = dp.tile([NTOK, 128], BF16, name="ckb_d")
            tqb_d = dp.tile([TTOK, 128], BF16, name="tqb_d")

            rs_tiles = {}
            rs2_tiles = {}

            def emit_group(g):
                """4 b's: dot matmuls -> exp -> rowsums -> alpha."""
                q = g // 8
                dps = dotp.tile([M, 128], F32, space="PSUM", tag="dot")
                for r in range(4):
                    b = 4 * g + r
                    nc.tensor.matmul(
                        dps[:, 32 * r:32 * (r + 1)],
                        cknT[0:TW, M * b:M * (b + 1)],
                        tqnT[0:TW, J * b:J * (b + 1)],
                        start=True, stop=True)
                sl = slice(128 * g, 128 * (g + 1))
                nc.scalar.activation(ET_all[:, sl], dps[:], AF.Exp)
                s = g // 4
                if s not in rs_tiles:
                    rs_tiles[s] = rsp.tile([1, 512], F32, space="PSUM",
                                           tag="rs", name=f"rs_{s}")
                nc.tensor.matmul(
                    rs_tiles[s][:, 128 * (g % 4):128 * (g % 4 + 1)],
                    ones100[:], ET_all[:, sl], start=True, stop=True)
                if q not in rs2_tiles:
                    rs2_tiles[q] = rs2p.tile([128, 8], F32, space="PSUM",
                                             tag="rs2", name=f"rs2_{q}")
                nc.tensor.matmul(rs2_tiles[q][:, (g % 8):(g % 8) + 1],
                                 ET_all[:, sl], ones100[:],
                                 start=True, stop=True)
                aps = alp.tile([E, 128], F32, space="PSUM", tag="alpha")
                for r in range(4):
                    b = 4 * g + r
                    nc.tensor.matmul(
                        aps[:, 32 * r:32 * (r + 1)],
                        u_all[:, b, :], ET_all[:, J * b:J * (b + 1)],
                        start=True, stop=True)
                nc.vector.tensor_copy(alphaT[:, sl], aps[:])

            def emit_s512(s):
                """512 tokens: z^T = W2 @ alphaT + b2 x rowsum -> zfinT."""
                sl = slice(512 * s, 512 * (s + 1))
                rs_row = wp.tile([1, 512], BF16, tag="rsrow")
                nc.vector.tensor_copy(rs_row[:], rs_tiles[s][:])
                zps = zpp.tile([E, 512], F32, space="PSUM", tag="z")
                nc.tensor.matmul(zps[:], w2t_t[:], alphaT[:, sl],
                                 start=True, stop=False)
                nc.tensor.matmul(zps[:], b2_t[:], rs_row[:],
                                 start=False, stop=True)
                nc.vector.tensor_copy(zfinT[:, sl], zps[:])

            def emit_zout(q):
                """XBAR transpose to token-major, divide by rowsum, store."""
                ztok = zp.tile([128, 8, E], BF16, tag="ztok")
                nc.sync.dma_start(out=ztok[:],
                                  in_=zfinT[:, 1024 * q:1024 * (q + 1)],
                                  transpose=True)
                ztok2 = zp.tile([128, 8, E], BF16, tag="ztok2")
                nc.vector.tensor_tensor(
                    out=ztok2[:], in0=ztok[:],
                    in1=inv_all[:, 8 * q:8 * (q + 1), None]
                        .to_broadcast([128, 8, E]),
                    op=OP.mult)
                nc.sync.dma_start(
                    out=zout[1024 * q:1024 * (q + 1), :].rearrange(
                        "(i p) e -> p i e", p=128),
                    in_=ztok2[:])

            for q in range(NCH):
                if q >= 1:
                    emit_zout(q - 1)
                # -- c gathers + bounce + transpose, SUB tiles at a time --
                for sgrp in range(TPC // SUB):
                    c_tile = cgp.tile([128, SUB, CE], BF16, tag="cg")
                    for i in range(SUB):
                        jt = TPC * q + SUB * sgrp + i
                        nc.gpsimd.indirect_dma_start(
                            out=c_tile[:, i, :], out_offset=None,
                            in_=cfull[:, :],
                            in_offset=bass.IndirectOffsetOnAxis(
                                ap=cidx_t[:, jt:jt + 1], axis=0))
                    r0 = 128 * (TPC * q + SUB * sgrp)
                    nrow = 128 * SUB
                    nc.sync.dma_start(
                        out=ub_d[r0:r0 + nrow, :].rearrange(
                            "(i p) e -> p i e", p=128),
                        in_=c_tile[:, :, 0:E])
                    nc.sync.dma_start(
                        out=ckb_d[r0:r0 + nrow, 0:DA].rearrange(
                            "(i p) e -> p i e", p=128),
                        in_=c_tile[:, :, E:CE])
                    nc.sync.dma_start(out=cknT[:, r0:r0 + nrow],
                                      in_=ckb_d[r0:r0 + nrow, :],
                                      transpose=True)
                # mask row for this chunk (after the transposes above)
                c0 = NTOK // NCH * q
                nc.scalar.dma_start(
                    out=cknT[DA:DA + 1, c0:c0 + NTOK // NCH],
                    in_=negm[:, c0:c0 + NTOK // NCH])
                # -- t gathers + bounce + transpose (8 tiles per chunk) --
                for th in range(2):
                    t_tile = tgp.tile([128, 4, TW], BF16, tag="tg")
                    for i in range(4):
                        st = 8 * q + 4 * th + i
                        nc.gpsimd.indirect_dma_start(
                            out=t_tile[:, i, :], out_offset=None,
                            in_=tfull[:, :],
                            in_offset=bass.IndirectOffsetOnAxis(
                                ap=tidx_t[:, st:st + 1], axis=0))
                    t0 = 128 * (8 * q + 4 * th)
                    nc.scalar.dma_start(
                        out=tqb_d[t0:t0 + 512, 0:TW].rearrange(
                            "(i p) e -> p i e", p=128),
                        in_=t_tile[:])
                    nc.scalar.dma_start(out=tqnT[:, t0:t0 + 512],
                                        in_=tqb_d[t0:t0 + 512, :],
                                        transpose=True)
                # -- u readback (m-part, per-b layout) --
                nc.scalar.dma_start(
                    out=u_all[:, BPC * q:BPC * (q + 1), :],
                    in_=ub_d[NTOK // NCH * q:NTOK // NCH * (q + 1), :]
                        .rearrange("(b m) e -> m b e", b=BPC))
                # -- compute: 8 groups of 4 b's, then per-512 endgame --
                for gg in range(8):
                    g = 8 * q + gg
                    emit_group(g)
                    if g % 4 == 3:
                        emit_s512(g // 4)
                nc.vector.reciprocal(inv_all[:, 8 * q:8 * (q + 1)],
                                     rs2_tiles[q][:])
            emit_zout(NCH - 1)

    nc.finalize()
    return nc


_nc_cache = [None]


def kernel(batch_titems, batch_citems, pad_rows, pad_cols, tvec, cvec,
           Ac_w, Ac_b, At_w, At_b, Bc_w, Bc_b, R_w, R_b):
    batch_titems = np.asarray(batch_titems).astype(np.int32)
    batch_citems = np.asarray(batch_citems).astype(np.int32)
    pad_rows = np.asarray(pad_rows).astype(np.int64)
    pad_cols = np.asarray(pad_cols).astype(np.int64)
    tvec = np.asarray(tvec, dtype=np.float32)
    cvec = np.asarray(cvec, dtype=np.float32)
    Ac_w = np.asarray(Ac_w, dtype=np.float32)
    Ac_b = np.asarray(Ac_b, dtype=np.float32)
    At_w = np.asarray(At_w, dtype=np.float32)
    At_b = np.asarray(At_b, dtype=np.float32)
    Bc_w = np.asarray(Bc_w, dtype=np.float32)
    Bc_b = np.asarray(Bc_b, dtype=np.float32)
    R_w = np.asarray(R_w, dtype=np.float32)
    R_b = np.asarray(R_b, dtype=np.float32)

    # ---- host folding: normalized projection tables, fused W2/b2 ----
    ck = cvec @ Ac_w.T + Ac_b                        # [V, 40]
    nck = np.maximum(np.linalg.norm(ck, axis=1, keepdims=True), 1e-6)
    cfull = np.empty((V, CE), dtype=ml_dtypes.bfloat16)
    cfull[:, :E] = cvec.astype(ml_dtypes.bfloat16)
    cfull[:, E:] = (ck / nck).astype(ml_dtypes.bfloat16)
    tq = tvec @ At_w.T + At_b                        # [V, 40]
    ntq = np.maximum(np.linalg.norm(tq, axis=1, keepdims=True), 1e-6)
    tfull = np.ones((V, TW), dtype=ml_dtypes.bfloat16)
    tfull[:, :DA] = (tq / ntq).astype(ml_dtypes.bfloat16)
    W2 = R_w @ Bc_w
    w2t = np.ascontiguousarray(W2.T).astype(ml_dtypes.bfloat16)
    b2 = (R_w @ Bc_b + R_b).astype(np.float32).reshape(1, E).astype(
        ml_dtypes.bfloat16)

    in_maps = []
    for c in range(NCORES):
        b0 = c * BL
        cit = batch_citems[b0:b0 + BL].ravel()       # [12800] b-major
        tit = batch_titems[b0:b0 + BL].ravel()       # [4096]
        cidx = np.ascontiguousarray(cit.reshape(NTC, 128).T.astype(np.int32))
        tidx = np.ascontiguousarray(tit.reshape(NTT, 128).T.astype(np.int32))
        sel = (pad_rows >= b0) & (pad_rows < b0 + BL)
        negm = np.zeros((1, NTOK), dtype=ml_dtypes.bfloat16)
        flat = (pad_rows[sel] - b0) * M + pad_cols[sel]
        negm[0, flat] = NEG
        in_maps.append({
            "cfull": cfull, "tfull": tfull,
            "cidx": cidx, "tidx": tidx, "negm": negm,
            "w2t": w2t, "b2r": b2,
        })

    if _nc_cache[0] is None:
        _nc_cache[0] = _build_bass()
    nc = _nc_cache[0]

    res = run_bass_kernel_spmd(nc, in_maps, list(range(NCORES)),
                               trace=_trace[0])
    _last_exec_ns[0] = res.exec_time_ns
    z = np.stack([r["zout"].astype(np.float32).reshape(BL, J, E)
                  for r in res.results], axis=0)
    return z.reshape(B, J, E)


# revision 6
# speedup vs baseline: 2.2574x; 1.3765x over previous
"""AttentiveItemToVec TRN2 kernel (8 NeuronCores, SPMD data-parallel over batch).

Math per batch row b (J=32 targets, M=100 contexts, E=128, DA=40):
  cos[j,m] = <tqn_j, ckn_m> with tqn/ckn the A-projected, per-VOCAB-normalized
             embeddings (norms are pure functions of the vocab row -> host).
  attn = softmax_m(cos + mask);  z = (attn @ u) @ W2^T + b2
             (W2 = R_w@Bc_w, b2 = R_w@Bc_b + R_b, using sum(attn)=1)

Device strategy per core (128 batch rows = 12800 c-tokens, 4096 t-tokens):
  - gather tables (host-precomputed, bf16):
      cfull [1M, 168] = [cvec | ckn],  tfull [1M, 41] = [tqn | 1.0]
  - 100 + 32 indirect row-gathers (128 rows each; the [P,1]-offset form is
    the only working indirect primitive, ~1.4us/instr on the gpsimd queue ->
    the wall-clock floor; everything else is hidden under the gather stream)
  - ALL transposes on the PE (tensor engine): XBAR DMA-transposes act as
    DMA-pipeline barriers and stall the gather descriptor stream ~5us each
  - pad mask folded into the dot matmul as contraction row 40 (lhsT row 40 =
    0/-1e30 per token, rhs row 40 = 1.0 from the table)
  - u m-major per-b layout via a DRAM bounce (direct DMAs only)
  - endgame: z^T = W2 @ alphaT per 512 tokens; PE-transpose back to
    token-major in 128-token blocks, x 1/rowsum (per-partition scalar),
    + b2 (replicated const tile), store bf16
"""
import sys

sys.path.insert(0, "/opt/trn_rl_repo")

import numpy as np
import ml_dtypes

import concourse.bass as bass
import concourse.mybir as mybir
from concourse import bacc
from concourse.tile import TileContext
from concourse.bass_utils import run_bass_kernel_spmd

F32 = mybir.dt.float32
BF16 = mybir.dt.bfloat16
I32 = mybir.dt.int32
AF = mybir.ActivationFunctionType
OP = mybir.AluOpType

V, E, DA = 1_000_000, 128, 40
B, J, M = 1024, 32, 100
NCORES = 8
BL = B // NCORES            # 128 batch rows per core
CE = E + DA                 # 168 fused c row: [u(128) | ckn(40)]
TW = DA + 1                 # 41 t row: [tqn(40) | 1.0]
NTC = BL * M // 128         # 100 c-gather tiles
NTT = BL * J // 128         # 32 t-gather tiles
NTOK = BL * M               # 12800 c tokens
TTOK = BL * J               # 4096 t tokens
NCH = 4                     # chunks of 32 b's
BPC = BL // NCH             # 32 b per chunk
TPC = NTC // NCH            # 25 c tiles per chunk
SUB = 5                     # c tiles per bounce sub-write
NEG = -1e30

_trace = [False]
_last_exec_ns = [None]


def _build_bass():
    nc = bacc.Bacc("TRN2", target_bir_lowering=False, debug=False,
                   num_devices=NCORES)

    cfull = nc.declare_dram_parameter("cfull", [V, CE], BF16, isOutput=False)
    tfull = nc.declare_dram_parameter("tfull", [V, TW], BF16, isOutput=False)
    cidx = nc.declare_dram_parameter("cidx", [128, NTC], I32, isOutput=False)
    tidx = nc.declare_dram_parameter("tidx", [128, NTT], I32, isOutput=False)
    negm = nc.declare_dram_parameter("negm", [1, NTOK], BF16, isOutput=False)
    w2t = nc.declare_dram_parameter("w2t", [E, E], BF16, isOutput=False)
    b2f = nc.declare_dram_parameter("b2f", [128, E], F32, isOutput=False)
    identd = nc.declare_dram_parameter("identd", [128, 128], BF16,
                                       isOutput=False)
    zout = nc.declare_dram_parameter("zout", [TTOK, E], BF16, isOutput=True)

    with TileContext(nc) as tc:
        with tc.tile_pool(name="const", bufs=1) as cp, \
             tc.tile_pool(name="big", bufs=1) as bigp, \
             tc.tile_pool(name="dram", bufs=1, space="DRAM") as dp, \
             tc.tile_pool(name="cg", bufs=8) as cgp, \
             tc.tile_pool(name="tg", bufs=4) as tgp, \
             tc.tile_pool(name="wrk", bufs=2) as wp, \
             tc.tile_pool(name="dotps", bufs=1, space="PSUM") as dotp, \
             tc.tile_pool(name="trps", bufs=2, space="PSUM") as trp, \
             tc.tile_pool(name="rs2ps", bufs=1, space="PSUM") as rs2p, \
             tc.tile_pool(name="zps", bufs=1, space="PSUM") as zpp, \
             tc.tile_pool(name="sqps", bufs=1, space="PSUM") as sqp:

            # ---- constants ----
            cidx_t = cp.tile([128, NTC], I32)
            nc.sync.dma_start(out=cidx_t[:], in_=cidx[:, :])
            tidx_t = cp.tile([128, NTT], I32)
            nc.sync.dma_start(out=tidx_t[:], in_=tidx[:, :])
            w2t_t = cp.tile([E, E], BF16)
            nc.sync.dma_start(out=w2t_t[:], in_=w2t[:, :])
            b2_t = cp.tile([128, E], F32)
            nc.sync.dma_start(out=b2_t[:], in_=b2f[:, :])
            ident = cp.tile([128, 128], BF16)
            nc.sync.dma_start(out=ident[:], in_=identd[:, :])
            ones100 = cp.tile([M, 1], BF16)
            nc.vector.memset(ones100[:], 1.0)

            # ---- persistent arrays ----
            cknT = bigp.tile([128, NTOK], BF16)      # rows 0:40 ckn, 40 mask
            tqnT = bigp.tile([128, TTOK], BF16)      # rows 0:40 tqn, 40 ones
            u_all = bigp.tile([M, BL, E], BF16)      # m-part, b, e
            ET_all = bigp.tile([M, TTOK], BF16)      # exp(cos+mask), m-part
            alphaT = bigp.tile([E, TTOK], BF16)      # E-major alpha (unnorm)
            inv_all = bigp.tile([128, NTT], F32)     # 1/rowsum, token-major

            ub_d = dp.tile([NTOK, E], BF16, name="ub_d")

            rs2_tiles = {}

            def emit_group(g):
                """4 b's: dot matmuls -> exp -> rowsum -> alpha."""
                q = g // 8
                dps = dotp.tile([M, 128], F32, space="PSUM", tag="dot")
                for r in range(4):
                    b = 4 * g + r
                    nc.tensor.matmul(
                        dps[:, 32 * r:32 * (r + 1)],
                        cknT[0:TW, M * b:M * (b + 1)],
                        tqnT[0:TW, J * b:J * (b + 1)],
                        start=True, stop=True)
                sl = slice(128 * g, 128 * (g + 1))
                nc.scalar.activation(ET_all[:, sl], dps[:], AF.Exp)
                if q not in rs2_tiles:
                    rs2_tiles[q] = rs2p.tile([128, 8], F32, space="PSUM",
                                             tag="rs2", name=f"rs2_{q}")
                nc.tensor.matmul(rs2_tiles[q][:, (g % 8):(g % 8) + 1],
                                 ET_all[:, sl], ones100[:],
                                 start=True, stop=True)
                aps = sqp.tile([E, 128], F32, space="PSUM", tag="sq", bufs=2)
                for r in range(4):
                    b = 4 * g + r
                    nc.tensor.matmul(
                        aps[:, 32 * r:32 * (r + 1)],
                        u_all[:, b, :], ET_all[:, J * b:J * (b + 1)],
                        start=True, stop=True)
                nc.vector.tensor_copy(alphaT[:, sl], aps[:])

            def emit_s512(s):
                """512 tokens: z^T = W2 @ alphaT; back to token-major; store."""
                sl = slice(512 * s, 512 * (s + 1))
                zps = zpp.tile([E, 512], F32, space="PSUM", tag="z")
                nc.tensor.matmul(zps[:], w2t_t[:], alphaT[:, sl],
                                 start=True, stop=True)
                zfin = wp.tile([E, 512], BF16, tag="zfin")
                nc.vector.tensor_copy(zfin[:], zps[:])
                for k in range(4):
                    tk = 4 * s + k            # global 128-token block
                    ztp = sqp.tile([128, 128], BF16, space="PSUM", tag="sqz",
                                   bufs=1, name=f"ztp_{tk}")
                    nc.tensor.transpose(ztp[:], zfin[:, 128 * k:128 * (k + 1)],
                                        ident[:])
                    zmul = wp.tile([128, E], F32, tag="zmul")
                    nc.vector.tensor_scalar_mul(zmul[:], ztp[:],
                                                inv_all[:, tk:tk + 1])
                    ztk = wp.tile([128, E], BF16, tag="ztk")
                    nc.vector.tensor_tensor(out=ztk[:], in0=zmul[:],
                                            in1=b2_t[:], op=OP.add)
                    nc.scalar.dma_start(out=zout[128 * tk:128 * (tk + 1), :],
                                        in_=ztk[:])

            for q in range(NCH):
                # -- c gathers; PE-transpose ck; bounce u --
                for sgrp in range(TPC // SUB):
                    c_tile = cgp.tile([128, SUB, CE], BF16, tag="cg")
                    for i in range(SUB):
                        jt = TPC * q + SUB * sgrp + i
                        nc.gpsimd.indirect_dma_start(
                            out=c_tile[:, i, :], out_offset=None,
                            in_=cfull[:, :],
                            in_offset=bass.IndirectOffsetOnAxis(
                                ap=cidx_t[:, jt:jt + 1], axis=0))
                    r0 = 128 * (TPC * q + SUB * sgrp)
                    nc.sync.dma_start(
                        out=ub_d[r0:r0 + 128 * SUB, :].rearrange(
                            "(i p) e -> p i e", p=128),
                        in_=c_tile[:, :, 0:E])
                    for i in range(SUB):
                        jt = TPC * q + SUB * sgrp + i
                        ckp = trp.tile([TW, 128], BF16, space="PSUM", tag="tr")
                        nc.tensor.transpose(ckp[0:DA, :],
                                            c_tile[:, i, E:CE], ident[:])
                        nc.vector.tensor_copy(
                            cknT[0:DA, 128 * jt:128 * (jt + 1)], ckp[0:DA, :])
                # mask row for this chunk
                c0 = NTOK // NCH * q
                nc.scalar.dma_start(
                    out=cknT[DA:DA + 1, c0:c0 + NTOK // NCH],
                    in_=negm[:, c0:c0 + NTOK // NCH])
                # -- t gathers; PE-transpose tq --
                for th in range(2):
                    t_tile = tgp.tile([128, 4, TW], BF16, tag="tg")
                    for i in range(4):
                        st = 8 * q + 4 * th + i
                        nc.gpsimd.indirect_dma_start(
                            out=t_tile[:, i, :], out_offset=None,
                            in_=tfull[:, :],
                            in_offset=bass.IndirectOffsetOnAxis(
                                ap=tidx_t[:, st:st + 1], axis=0))
                    for i in range(4):
                        st = 8 * q + 4 * th + i
                        tqp = trp.tile([TW, 128], BF16, space="PSUM", tag="tr")
                        nc.tensor.transpose(tqp[:], t_tile[:, i, :], ident[:])
                        nc.vector.tensor_copy(
                            tqnT[0:TW, 128 * st:128 * (st + 1)], tqp[:])
                # -- u readback (m-part, per-b layout) --
                nc.scalar.dma_start(
                    out=u_all[:, BPC * q:BPC * (q + 1), :],
                    in_=ub_d[NTOK // NCH * q:NTOK // NCH * (q + 1), :]
                        .rearrange("(b m) e -> m b e", b=BPC))
                # -- compute: 8 groups of 4 b's; recip; endgame --
                for gg in range(8):
                    emit_group(8 * q + gg)
                nc.vector.reciprocal(inv_all[:, 8 * q:8 * (q + 1)],
                                     rs2_tiles[q][:])
                emit_s512(2 * q)
                emit_s512(2 * q + 1)

    nc.finalize()
    return nc


_nc_cache = [None]


def kernel(batch_titems, batch_citems, pad_rows, pad_cols, tvec, cvec,
           Ac_w, Ac_b, At_w, At_b, Bc_w, Bc_b, R_w, R_b):
    batch_titems = np.asarray(batch_titems).astype(np.int32)
    batch_citems = np.asarray(batch_citems).astype(np.int32)
    pad_rows = np.asarray(pad_rows).astype(np.int64)
    pad_cols = np.asarray(pad_cols).astype(np.int64)
    tvec = np.asarray(tvec, dtype=np.float32)
    cvec = np.asarray(cvec, dtype=np.float32)
    Ac_w = np.asarray(Ac_w, dtype=np.float32)
    Ac_b = np.asarray(Ac_b, dtype=np.float32)
    At_w = np.asarray(At_w, dtype=np.float32)
    At_b = np.asarray(At_b, dtype=np.float32)
    Bc_w = np.asarray(Bc_w, dtype=np.float32)
    Bc_b = np.asarray(Bc_b, dtype=np.float32)
    R_w = np.asarray(R_w, dtype=np.float32)
    R_b = np.asarray(R_b, dtype=np.float32)

    # ---- host folding: normalized projection tables, fused W2/b2 ----
    ck = cvec @ Ac_w.T + Ac_b                        # [V, 40]
    nck = np.maximum(np.linalg.norm(ck, axis=1, keepdims=True), 1e-6)
    cfull = np.empty((V, CE), dtype=ml_dtypes.bfloat16)
    cfull[:, :E] = cvec.astype(ml_dtypes.bfloat16)
    cfull[:, E:] = (ck / nck).astype(ml_dtypes.bfloat16)
    tq = tvec @ At_w.T + At_b                        # [V, 40]
    ntq = np.maximum(np.linalg.norm(tq, axis=1, keepdims=True), 1e-6)
    tfull = np.ones((V, TW), dtype=ml_dtypes.bfloat16)
    tfull[:, :DA] = (tq / ntq).astype(ml_dtypes.bfloat16)
    W2 = R_w @ Bc_w
    w2t = np.ascontiguousarray(W2.T).astype(ml_dtypes.bfloat16)
    b2 = (R_w @ Bc_b + R_b).astype(np.float32)
    b2f = np.ascontiguousarray(np.broadcast_to(b2, (128, E)))
    identd = np.eye(128, dtype=np.float32).astype(ml_dtypes.bfloat16)

    in_maps = []
    for c in range(NCORES):
        b0 = c * BL
        cit = batch_citems[b0:b0 + BL].ravel()       # [12800] b-major
        tit = batch_titems[b0:b0 + BL].ravel()       # [4096]
        cidx = np.ascontiguousarray(cit.reshape(NTC, 128).T.astype(np.int32))
        tidx = np.ascontiguousarray(tit.reshape(NTT, 128).T.astype(np.int32))
        sel = (pad_rows >= b0) & (pad_rows < b0 + BL)
        negm = np.zeros((1, NTOK), dtype=ml_dtypes.bfloat16)
        flat = (pad_rows[sel] - b0) * M + pad_cols[sel]
        negm[0, flat] = NEG
        in_maps.append({
            "cfull": cfull, "tfull": tfull,
            "cidx": cidx, "tidx": tidx, "negm": negm,
            "w2t": w2t, "b2f": b2f, "identd": identd,
        })

    if _nc_cache[0] is None:
        _nc_cache[0] = _build_bass()
    nc = _nc_cache[0]

    res = run_bass_kernel_spmd(nc, in_maps, list(range(NCORES)),
                               trace=_trace[0])
    _last_exec_ns[0] = res.exec_time_ns
    z = np.stack([r["zout"].astype(np.float32).reshape(BL, J, E)
                  for r in res.results], axis=0)
    return z.reshape(B, J, E)


# revision 7
# speedup vs baseline: 2.3331x; 1.0336x over previous
"""AttentiveItemToVec TRN2 kernel (8 NeuronCores, SPMD data-parallel over batch).

Math per batch row b (J=32 targets, M=100 contexts, E=128, DA=40):
  cos[j,m] = <tqn_j, ckn_m> with tqn/ckn the A-projected, per-VOCAB-normalized
             embeddings (norms are pure functions of the vocab row -> host).
  attn = softmax_m(cos + mask);  z = (attn @ u) @ W2^T + b2
             (W2 = R_w@Bc_w, b2 = R_w@Bc_b + R_b, using sum(attn)=1)

Device strategy per core (128 batch rows = 12800 c-tokens, 4096 t-tokens):
  - gather tables (host-precomputed, bf16):
      cfull [1M, 168] = [cvec | ckn],  tfull [1M, 41] = [tqn | 1.0]
  - 100 + 32 indirect row-gathers (128 rows each; the [P,1]-offset form is
    the only working indirect primitive, ~1.4us/instr on the gpsimd queue ->
    the wall-clock floor; everything else is hidden under the gather stream)
  - ALL transposes on the PE (tensor engine): XBAR DMA-transposes act as
    DMA-pipeline barriers and stall the gather descriptor stream ~5us each
  - pad mask folded into the dot matmul as contraction row 40 (lhsT row 40 =
    0/-1e30 per token, rhs row 40 = 1.0 from the table)
  - u m-major per-b layout via a DRAM bounce (direct DMAs only), read back
    per 4-b group so compute pipelines at group granularity (short tail)
  - endgame: z^T = W2 @ alphaT per 512 tokens; PE-transpose back to
    token-major in 128-token blocks, x 1/rowsum (per-partition scalar),
    + b2 (replicated const tile), store bf16
"""
import sys

sys.path.insert(0, "/opt/trn_rl_repo")

import numpy as np
import ml_dtypes

import concourse.bass as bass
import concourse.mybir as mybir
from concourse import bacc
from concourse.tile import TileContext
from concourse.bass_utils import run_bass_kernel_spmd

F32 = mybir.dt.float32
BF16 = mybir.dt.bfloat16
I32 = mybir.dt.int32
AF = mybir.ActivationFunctionType
OP = mybir.AluOpType

V, E, DA = 1_000_000, 128, 40
B, J, M = 1024, 32, 100
NCORES = 8
BL = B // NCORES            # 128 batch rows per core
CE = E + DA                 # 168 fused c row: [u(128) | ckn(40)]
TW = DA + 1                 # 41 t row: [tqn(40) | 1.0]
NTC = BL * M // 128         # 100 c-gather tiles
NTT = BL * J // 128         # 32 t-gather tiles
NTOK = BL * M               # 12800 c tokens
TTOK = BL * J               # 4096 t tokens
NCH = 4                     # chunks of 32 b's
TPC = NTC // NCH            # 25 c tiles per chunk
SUB = 5                     # c tiles per bounce sub-write
NEG = -1e30

_trace = [False]
_last_exec_ns = [None]


def _build_bass():
    nc = bacc.Bacc("TRN2", target_bir_lowering=False, debug=False,
                   num_devices=NCORES)

    cfull = nc.declare_dram_parameter("cfull", [V, CE], BF16, isOutput=False)
    tfull = nc.declare_dram_parameter("tfull", [V, TW], BF16, isOutput=False)
    cidx = nc.declare_dram_parameter("cidx", [128, NTC], I32, isOutput=False)
    tidx = nc.declare_dram_parameter("tidx", [128, NTT], I32, isOutput=False)
    negm = nc.declare_dram_parameter("negm", [1, NTOK], BF16, isOutput=False)
    w2t = nc.declare_dram_parameter("w2t", [E, E], BF16, isOutput=False)
    b2f = nc.declare_dram_parameter("b2f", [128, E], F32, isOutput=False)
    identd = nc.declare_dram_parameter("identd", [128, 128], BF16,
                                       isOutput=False)
    zout = nc.declare_dram_parameter("zout", [TTOK, E], BF16, isOutput=True)

    with TileContext(nc) as tc:
        with tc.tile_pool(name="const", bufs=1) as cp, \
             tc.tile_pool(name="big", bufs=1) as bigp, \
             tc.tile_pool(name="dram", bufs=1, space="DRAM") as dp, \
             tc.tile_pool(name="cg", bufs=8) as cgp, \
             tc.tile_pool(name="tg", bufs=4) as tgp, \
             tc.tile_pool(name="wrk", bufs=2) as wp, \
             tc.tile_pool(name="dotps", bufs=1, space="PSUM") as dotp, \
             tc.tile_pool(name="trps", bufs=1, space="PSUM") as trp, \
             tc.tile_pool(name="rs2ps", bufs=1, space="PSUM") as rs2p, \
             tc.tile_pool(name="zpsp", bufs=1, space="PSUM") as zpp, \
             tc.tile_pool(name="sqps", bufs=1, space="PSUM") as sqp:

            # ---- constants ----
            cidx_t = cp.tile([128, NTC], I32)
            nc.sync.dma_start(out=cidx_t[:], in_=cidx[:, :])
            tidx_t = cp.tile([128, NTT], I32)
            nc.sync.dma_start(out=tidx_t[:], in_=tidx[:, :])
            w2t_t = cp.tile([E, E], BF16)
            nc.sync.dma_start(out=w2t_t[:], in_=w2t[:, :])
            b2_t = cp.tile([128, E], F32)
            nc.sync.dma_start(out=b2_t[:], in_=b2f[:, :])
            ident = cp.tile([128, 128], BF16)
            nc.sync.dma_start(out=ident[:], in_=identd[:, :])
            ones100 = cp.tile([M, 1], BF16)
            nc.vector.memset(ones100[:], 1.0)

            # ---- persistent arrays ----
            cknT = bigp.tile([128, NTOK], BF16)      # rows 0:40 ckn, 40 mask
            tqnT = bigp.tile([128, TTOK], BF16)      # rows 0:40 tqn, 40 ones
            u_all = bigp.tile([M, BL, E], BF16)      # m-part, b, e
            ET_all = bigp.tile([M, TTOK], BF16)      # exp(cos+mask), m-part
            alphaT = bigp.tile([E, TTOK], BF16)      # E-major alpha (unnorm)
            inv_all = bigp.tile([128, NTT], F32)     # 1/rowsum, token-major

            ub_d = dp.tile([NTOK, E], BF16, name="ub_d")

            rs2_tiles = {}

            def emit_group(g):
                """4 b's: u readback -> dot matmuls -> exp -> rowsum -> alpha."""
                nc.sync.dma_start(
                    out=u_all[:, 4 * g:4 * (g + 1), :],
                    in_=ub_d[400 * g:400 * (g + 1), :]
                        .rearrange("(b m) e -> m b e", b=4))
                dps = dotp.tile([M, 128], F32, space="PSUM", tag="dot")
                for r in range(4):
                    b = 4 * g + r
                    nc.tensor.matmul(
                        dps[:, 32 * r:32 * (r + 1)],
                        cknT[0:TW, M * b:M * (b + 1)],
                        tqnT[0:TW, J * b:J * (b + 1)],
                        start=True, stop=True)
                sl = slice(128 * g, 128 * (g + 1))
                nc.scalar.activation(ET_all[:, sl], dps[:], AF.Exp)
                s = g // 4
                if s not in rs2_tiles:
                    rs2_tiles[s] = rs2p.tile([128, 4], F32, space="PSUM",
                                             tag="rs2", name=f"rs2_{s}")
                nc.tensor.matmul(rs2_tiles[s][:, (g % 4):(g % 4) + 1],
                                 ET_all[:, sl], ones100[:],
                                 start=True, stop=True)
                aps = sqp.tile([E, 128], F32, space="PSUM", tag="sq", bufs=2)
                for r in range(4):
                    b = 4 * g + r
                    nc.tensor.matmul(
                        aps[:, 32 * r:32 * (r + 1)],
                        u_all[:, b, :], ET_all[:, J * b:J * (b + 1)],
                        start=True, stop=True)
                nc.vector.tensor_copy(alphaT[:, sl], aps[:])

            def emit_s512(s):
                """512 tokens: recip; z^T = W2 @ alphaT; token-major; store."""
                nc.vector.reciprocal(inv_all[:, 4 * s:4 * (s + 1)],
                                     rs2_tiles[s][:])
                sl = slice(512 * s, 512 * (s + 1))
                zps = zpp.tile([E, 512], F32, space="PSUM", tag="z")
                nc.tensor.matmul(zps[:], w2t_t[:], alphaT[:, sl],
                                 start=True, stop=True)
                zfin = wp.tile([E, 512], BF16, tag="zfin")
                nc.vector.tensor_copy(zfin[:], zps[:])
                for k in range(4):
                    tk = 4 * s + k            # global 128-token block
                    ztp = sqp.tile([128, 128], BF16, space="PSUM", tag="sqz",
                                   bufs=1, name=f"ztp_{tk}")
                    nc.tensor.transpose(ztp[:], zfin[:, 128 * k:128 * (k + 1)],
                                        ident[:])
                    zmul = wp.tile([128, E], F32, tag="zmul")
                    nc.vector.tensor_scalar_mul(zmul[:], ztp[:],
                                                inv_all[:, tk:tk + 1])
                    ztk = wp.tile([128, E], BF16, tag="ztk")
                    nc.vector.tensor_tensor(out=ztk[:], in0=zmul[:],
                                            in1=b2_t[:], op=OP.add)
                    nc.scalar.dma_start(out=zout[128 * tk:128 * (tk + 1), :],
                                        in_=ztk[:])

            for q in range(NCH):
                # mask row for this chunk (row 40; disjoint from ck copies)
                c0 = NTOK // NCH * q
                nc.scalar.dma_start(
                    out=cknT[DA:DA + 1, c0:c0 + NTOK // NCH],
                    in_=negm[:, c0:c0 + NTOK // NCH])
                # -- t gathers + PE transposes (batched into one PSUM tile) --
                for th in range(2):
                    t_tile = tgp.tile([128, 4, TW], BF16, tag="tg")
                    for i in range(4):
                        st = 8 * q + 4 * th + i
                        nc.gpsimd.indirect_dma_start(
                            out=t_tile[:, i, :], out_offset=None,
                            in_=tfull[:, :],
                            in_offset=bass.IndirectOffsetOnAxis(
                                ap=tidx_t[:, st:st + 1], axis=0))
                    tqp = trp.tile([TW, 128 * SUB], BF16, space="PSUM",
                                   tag="tr")
                    for i in range(4):
                        nc.tensor.transpose(tqp[:, 128 * i:128 * (i + 1)],
                                            t_tile[:, i, :], ident[:])
                    t0 = 128 * (8 * q + 4 * th)
                    nc.vector.tensor_copy(tqnT[0:TW, t0:t0 + 512],
                                          tqp[:, 0:512])
                # -- c gathers; u bounce; batched ck transposes; compute --
                glast = 8 * q  # next group to emit
                for sgrp in range(TPC // SUB):
                    c_tile = cgp.tile([128, SUB, CE], BF16, tag="cg")
                    for i in range(SUB):
                        jt = TPC * q + SUB * sgrp + i
                        nc.gpsimd.indirect_dma_start(
                            out=c_tile[:, i, :], out_offset=None,
                            in_=cfull[:, :],
                            in_offset=bass.IndirectOffsetOnAxis(
                                ap=cidx_t[:, jt:jt + 1], axis=0))
                    r0 = 128 * (TPC * q + SUB * sgrp)
                    nc.sync.dma_start(
                        out=ub_d[r0:r0 + 128 * SUB, :].rearrange(
                            "(i p) e -> p i e", p=128),
                        in_=c_tile[:, :, 0:E])
                    ckp = trp.tile([TW, 128 * SUB], BF16, space="PSUM",
                                   tag="tr")
                    for i in range(SUB):
                        nc.tensor.transpose(ckp[0:DA, 128 * i:128 * (i + 1)],
                                            c_tile[:, i, E:CE], ident[:])
                    nc.vector.tensor_copy(
                        cknT[0:DA, r0:r0 + 128 * SUB], ckp[0:DA, :])
                    # groups fully covered by tokens gathered so far
                    tok_done = 640 * (sgrp + 1)
                    while glast < 8 * (q + 1) and \
                            400 * (glast - 8 * q + 1) <= tok_done:
                        emit_group(glast)
                        if glast % 4 == 3:
                            emit_s512(glast // 4)
                        glast += 1
                while glast < 8 * (q + 1):
                    emit_group(glast)
                    if glast % 4 == 3:
                        emit_s512(glast // 4)
                    glast += 1

    nc.finalize()
    return nc


_nc_cache = [None]


def kernel(batch_titems, batch_citems, pad_rows, pad_cols, tvec, cvec,
           Ac_w, Ac_b, At_w, At_b, Bc_w, Bc_b, R_w, R_b):
    batch_titems = np.asarray(batch_titems).astype(np.int32)
    batch_citems = np.asarray(batch_citems).astype(np.int32)
    pad_rows = np.asarray(pad_rows).astype(np.int64)
    pad_cols = np.asarray(pad_cols).astype(np.int64)
    tvec = np.asarray(tvec, dtype=np.float32)
    cvec = np.asarray(cvec, dtype=np.float32)
    Ac_w = np.asarray(Ac_w, dtype=np.float32)
    Ac_b = np.asarray(Ac_b, dtype=np.float32)
    At_w = np.asarray(At_w, dtype=np.float32)
    At_b = np.asarray(At_b, dtype=np.float32)
    Bc_w = np.asarray(Bc_w, dtype=np.float32)
    Bc_b = np.asarray(Bc_b, dtype=np.float32)
    R_w = np.asarray(R_w, dtype=np.float32)
    R_b = np.asarray(R_b, dtype=np.float32)

    # ---- host folding: normalized projection tables, fused W2/b2 ----
    ck = cvec @ Ac_w.T + Ac_b                        # [V, 40]
    nck = np.maximum(np.linalg.norm(ck, axis=1, keepdims=True), 1e-6)
    cfull = np.empty((V, CE), dtype=ml_dtypes.bfloat16)
    cfull[:, :E] = cvec.astype(ml_dtypes.bfloat16)
    cfull[:, E:] = (ck / nck).astype(ml_dtypes.bfloat16)
    tq = tvec @ At_w.T + At_b                        # [V, 40]
    ntq = np.maximum(np.linalg.norm(tq, axis=1, keepdims=True), 1e-6)
    tfull = np.ones((V, TW), dtype=ml_dtypes.bfloat16)
    tfull[:, :DA] = (tq / ntq).astype(ml_dtypes.bfloat16)
    W2 = R_w @ Bc_w
    w2t = np.ascontiguousarray(W2.T).astype(ml_dtypes.bfloat16)
    b2 = (R_w @ Bc_b + R_b).astype(np.float32)
    b2f = np.ascontiguousarray(np.broadcast_to(b2, (128, E)))
    identd = np.eye(128, dtype=np.float32).astype(ml_dtypes.bfloat16)

    in_maps = []
    for c in range(NCORES):
        b0 = c * BL
        cit = batch_citems[b0:b0 + BL].ravel()       # [12800] b-major
        tit = batch_titems[b0:b0 + BL].ravel()       # [4096]
        cidx = np.ascontiguousarray(cit.reshape(NTC, 128).T.astype(np.int32))
        tidx = np.ascontiguousarray(tit.reshape(NTT, 128).T.astype(np.int32))
        sel = (pad_rows >= b0) & (pad_rows < b0 + BL)
        negm = np.zeros((1, NTOK), dtype=ml_dtypes.bfloat16)
        flat = (pad_rows[sel] - b0) * M + pad_cols[sel]
        negm[0, flat] = NEG
        in_maps.append({
            "cfull": cfull, "tfull": tfull,
            "cidx": cidx, "tidx": tidx, "negm": negm,
            "w2t": w2t, "b2f": b2f, "identd": identd,
        })

    if _nc_cache[0] is None:
        _nc_cache[0] = _build_bass()
    nc = _nc_cache[0]

    res = run_bass_kernel_spmd(nc, in_maps, list(range(NCORES)),
                               trace=_trace[0])
    _last_exec_ns[0] = res.exec_time_ns
    z = np.stack([r["zout"].astype(np.float32).reshape(BL, J, E)
                  for r in res.results], axis=0)
    return z.reshape(B, J, E)


# revision 9
# speedup vs baseline: 2.7681x; 1.1864x over previous
"""AttentiveItemToVec TRN2 kernel (8 NeuronCores, SPMD data-parallel over batch).

Math per batch row b (J=32 targets, M=100 contexts, E=128, DA=40):
  cos[j,m] = <tqn_j, ckn_m> with tqn/ckn the A-projected, per-VOCAB-normalized
             embeddings (norms are pure functions of the vocab row -> host).
  attn = softmax_m(cos + mask);  z = (attn @ u) @ W2^T + b2
             (W2 = R_w@Bc_w, b2 = R_w@Bc_b + R_b, using sum(attn)=1)

Device strategy per core (128 batch rows = 12800 c-tokens, 4096 t-tokens):
  - gather tables (host-precomputed, bf16):
      cfull [1M, 168] = [cvec | ckn],  tfull [1M, 41] = [tqn | 1.0]
  - 100 + 32 indirect row-gathers (128 rows each; the [P,1]-offset form is
    the only working indirect primitive, ~1.4us/instr on the gpsimd queue ->
    the wall-clock floor; everything else is hidden under the gather stream)
  - ALL transposes on the PE (tensor engine): XBAR DMA-transposes act as
    DMA-pipeline barriers and stall the gather descriptor stream ~5us each
  - pad mask folded into the dot matmul as contraction row 40 (lhsT row 40 =
    0/-1e30 per token, rhs row 40 = 1.0 from the table)
  - u m-major per-b layout via a DRAM bounce (direct DMAs; PE matmul
    operands must sit at base partition 0/32/64, so reading u straight out
    of the token-major gather tiles is not expressible)
  - endgame per 4-b group (128 tokens): z^T = W2 @ alphaT; PE-transpose to
    token-major, x 1/rowsum (per-partition scalar), + b2 (replicated const
    tile), store bf16
"""
import sys

sys.path.insert(0, "/opt/trn_rl_repo")

import numpy as np
import ml_dtypes

import concourse.bass as bass
import concourse.mybir as mybir
from concourse import bacc
from concourse.tile import TileContext
from concourse.bass_utils import run_bass_kernel_spmd

F32 = mybir.dt.float32
BF16 = mybir.dt.bfloat16
I32 = mybir.dt.int32
AF = mybir.ActivationFunctionType
OP = mybir.AluOpType

V, E, DA = 1_000_000, 128, 40
B, J, M = 1024, 32, 100
NCORES = 8
BL = B // NCORES            # 128 batch rows per core
CE = E + DA                 # 168 fused c row: [u(128) | ckn(40)]
TW = DA + 1                 # 41 t row: [tqn(40) | 1.0]
NTC = BL * M // 128         # 100 c-gather tiles
NTT = BL * J // 128         # 32 t-gather tiles
NTOK = BL * M               # 12800 c tokens
TTOK = BL * J               # 4096 t tokens
NCH = 4                     # chunks of 32 b's
TPC = NTC // NCH            # 25 c tiles per chunk
SUB = 5                     # c tiles per bounce sub-write
NEG = -1e30

_trace = [False]
_last_exec_ns = [None]


def _build_bass():
    nc = bacc.Bacc("TRN2", target_bir_lowering=False, debug=False,
                   num_devices=NCORES)

    cfull = nc.declare_dram_parameter("cfull", [V, CE], BF16, isOutput=False)
    tfull = nc.declare_dram_parameter("tfull", [V, TW], BF16, isOutput=False)
    cidx = nc.declare_dram_parameter("cidx", [128, NTC], I32, isOutput=False)
    tidx = nc.declare_dram_parameter("tidx", [128, NTT], I32, isOutput=False)
    negm = nc.declare_dram_parameter("negm", [1, NTOK], BF16, isOutput=False)
    w2t = nc.declare_dram_parameter("w2t", [E, E], BF16, isOutput=False)
    b2f = nc.declare_dram_parameter("b2f", [128, E], F32, isOutput=False)
    identd = nc.declare_dram_parameter("identd", [128, 128], BF16,
                                       isOutput=False)
    zout = nc.declare_dram_parameter("zout", [TTOK, E], BF16, isOutput=True)

    with TileContext(nc) as tc:
        with tc.tile_pool(name="const", bufs=1) as cp, \
             tc.tile_pool(name="big", bufs=1) as bigp, \
             tc.tile_pool(name="dram", bufs=1, space="DRAM") as dp, \
             tc.tile_pool(name="cg", bufs=8) as cgp, \
             tc.tile_pool(name="tg", bufs=4) as tgp, \
             tc.tile_pool(name="wrk", bufs=2) as wp, \
             tc.tile_pool(name="dotps", bufs=1, space="PSUM") as dotp, \
             tc.tile_pool(name="trps", bufs=1, space="PSUM") as trp, \
             tc.tile_pool(name="rs2ps", bufs=1, space="PSUM") as rs2p, \
             tc.tile_pool(name="zpsp", bufs=1, space="PSUM") as zpp, \
             tc.tile_pool(name="sqps", bufs=1, space="PSUM") as sqp:

            # ---- constants ----
            cidx_t = cp.tile([128, NTC], I32)
            nc.sync.dma_start(out=cidx_t[:], in_=cidx[:, :])
            tidx_t = cp.tile([128, NTT], I32)
            nc.sync.dma_start(out=tidx_t[:], in_=tidx[:, :])
            w2t_t = cp.tile([E, E], BF16)
            nc.sync.dma_start(out=w2t_t[:], in_=w2t[:, :])
            b2_t = cp.tile([128, E], F32)
            nc.sync.dma_start(out=b2_t[:], in_=b2f[:, :])
            ident = cp.tile([128, 128], BF16)
            nc.sync.dma_start(out=ident[:], in_=identd[:, :])
            ones100 = cp.tile([M, 1], BF16)
            nc.vector.memset(ones100[:], 1.0)

            # ---- persistent arrays ----
            cknT = bigp.tile([128, NTOK], BF16)      # rows 0:40 ckn, 40 mask
            tqnT = bigp.tile([128, TTOK], BF16)      # rows 0:40 tqn, 40 ones
            ET_all = bigp.tile([M, TTOK], BF16)      # exp(cos+mask), m-part
            alphaT = bigp.tile([E, TTOK], BF16)      # E-major alpha (unnorm)
            inv_all = bigp.tile([128, NTT], F32)     # 1/rowsum, token-major
            u_all = bigp.tile([M, BL, E], BF16)      # m-part, b, e

            ub_d = dp.tile([NTOK, E], BF16, name="ub_d")

            def emit_group(g):
                """4 b's: dot -> exp -> rowsum -> alpha -> endgame -> store."""
                nc.sync.dma_start(
                    out=u_all[:, 4 * g:4 * (g + 1), :],
                    in_=ub_d[400 * g:400 * (g + 1), :]
                        .rearrange("(b m) e -> m b e", b=4))
                dps = dotp.tile([M, 128], F32, space="PSUM", tag="dot")
                for r in range(4):
                    b = 4 * g + r
                    nc.tensor.matmul(
                        dps[:, 32 * r:32 * (r + 1)],
                        cknT[0:TW, M * b:M * (b + 1)],
                        tqnT[0:TW, J * b:J * (b + 1)],
                        start=True, stop=True)
                sl = slice(128 * g, 128 * (g + 1))
                nc.scalar.activation(ET_all[:, sl], dps[:], AF.Exp)
                rs2 = rs2p.tile([128, 1], F32, space="PSUM", tag="rs2")
                nc.tensor.matmul(rs2[:], ET_all[:, sl], ones100[:],
                                 start=True, stop=True)
                nc.vector.reciprocal(inv_all[:, g:g + 1], rs2[:])
                aps = sqp.tile([E, 128], F32, space="PSUM", tag="sq", bufs=2)
                for r in range(4):
                    b = 4 * g + r
                    nc.tensor.matmul(
                        aps[:, 32 * r:32 * (r + 1)],
                        u_all[:, b, :], ET_all[:, J * b:J * (b + 1)],
                        start=True, stop=True)
                nc.vector.tensor_copy(alphaT[:, sl], aps[:])
                # endgame for these 128 tokens
                zps = zpp.tile([E, 128], F32, space="PSUM", tag="z")
                nc.tensor.matmul(zps[:], w2t_t[:], alphaT[:, sl],
                                 start=True, stop=True)
                zfin = wp.tile([E, 128], BF16, tag="zfin")
                nc.vector.tensor_copy(zfin[:], zps[:])
                ztp = sqp.tile([128, 128], BF16, space="PSUM", tag="sqz",
                               bufs=1, name=f"ztp_{g}")
                nc.tensor.transpose(ztp[:], zfin[:], ident[:])
                zmul = wp.tile([128, E], F32, tag="zmul")
                nc.vector.tensor_scalar_mul(zmul[:], ztp[:],
                                            inv_all[:, g:g + 1])
                ztk = wp.tile([128, E], BF16, tag="ztk")
                nc.vector.tensor_tensor(out=ztk[:], in0=zmul[:],
                                        in1=b2_t[:], op=OP.add)
                nc.sync.dma_start(out=zout[128 * g:128 * (g + 1), :],
                                  in_=ztk[:])

            for q in range(NCH):
                # mask row for this chunk (row 40; disjoint from ck copies)
                c0 = NTOK // NCH * q
                nc.scalar.dma_start(
                    out=cknT[DA:DA + 1, c0:c0 + NTOK // NCH],
                    in_=negm[:, c0:c0 + NTOK // NCH])
                # -- t gathers + PE transposes (batched into one PSUM tile) --
                for th in range(2):
                    t_tile = tgp.tile([128, 4, TW], BF16, tag="tg")
                    for i in range(4):
                        st = 8 * q + 4 * th + i
                        nc.gpsimd.indirect_dma_start(
                            out=t_tile[:, i, :], out_offset=None,
                            in_=tfull[:, :],
                            in_offset=bass.IndirectOffsetOnAxis(
                                ap=tidx_t[:, st:st + 1], axis=0))
                    tqp = trp.tile([TW, 128 * SUB], BF16, space="PSUM",
                                   tag="tr")
                    for i in range(4):
                        nc.tensor.transpose(tqp[:, 128 * i:128 * (i + 1)],
                                            t_tile[:, i, :], ident[:])
                    t0 = 128 * (8 * q + 4 * th)
                    nc.vector.tensor_copy(tqnT[0:TW, t0:t0 + 512],
                                          tqp[:, 0:512])
                # -- c gathers; batched ck transposes; compute per group --
                glast = 8 * q  # next group to emit
                for sgrp in range(TPC // SUB):
                    c_tile = cgp.tile([128, SUB, CE], BF16, tag="cg")
                    for i in range(SUB):
                        jt = TPC * q + SUB * sgrp + i
                        nc.gpsimd.indirect_dma_start(
                            out=c_tile[:, i, :], out_offset=None,
                            in_=cfull[:, :],
                            in_offset=bass.IndirectOffsetOnAxis(
                                ap=cidx_t[:, jt:jt + 1], axis=0))
                    r0 = 128 * (TPC * q + SUB * sgrp)
                    nc.sync.dma_start(
                        out=ub_d[r0:r0 + 128 * SUB, :].rearrange(
                            "(i p) e -> p i e", p=128),
                        in_=c_tile[:, :, 0:E])
                    ckp = trp.tile([TW, 128 * SUB], BF16, space="PSUM",
                                   tag="tr")
                    for i in range(SUB):
                        nc.tensor.transpose(ckp[0:DA, 128 * i:128 * (i + 1)],
                                            c_tile[:, i, E:CE], ident[:])
                    nc.vector.tensor_copy(
                        cknT[0:DA, r0:r0 + 128 * SUB], ckp[0:DA, :])
                    # groups fully covered by tokens gathered so far
                    tok_done = 640 * (sgrp + 1)
                    while glast < 8 * (q + 1) and \
                            400 * (glast - 8 * q + 1) <= tok_done:
                        emit_group(glast)
                        glast += 1
                while glast < 8 * (q + 1):
                    emit_group(glast)
                    glast += 1

    nc.finalize()
    return nc


_nc_cache = [None]


def kernel(batch_titems, batch_citems, pad_rows, pad_cols, tvec, cvec,
           Ac_w, Ac_b, At_w, At_b, Bc_w, Bc_b, R_w, R_b):
    batch_titems = np.asarray(batch_titems).astype(np.int32)
    batch_citems = np.asarray(batch_citems).astype(np.int32)
    pad_rows = np.asarray(pad_rows).astype(np.int64)
    pad_cols = np.asarray(pad_cols).astype(np.int64)
    tvec = np.asarray(tvec, dtype=np.float32)
    cvec = np.asarray(cvec, dtype=np.float32)
    Ac_w = np.asarray(Ac_w, dtype=np.float32)
    Ac_b = np.asarray(Ac_b, dtype=np.float32)
    At_w = np.asarray(At_w, dtype=np.float32)
    At_b = np.asarray(At_b, dtype=np.float32)
    Bc_w = np.asarray(Bc_w, dtype=np.float32)
    Bc_b = np.asarray(Bc_b, dtype=np.float32)
    R_w = np.asarray(R_w, dtype=np.float32)
    R_b = np.asarray(R_b, dtype=np.float32)

    # ---- host folding: normalized projection tables, fused W2/b2 ----
    ck = cvec @ Ac_w.T + Ac_b                        # [V, 40]
    nck = np.maximum(np.linalg.norm(ck, axis=1, keepdims=True), 1e-6)
    cfull = np.empty((V, CE), dtype=ml_dtypes.bfloat16)
    cfull[:, :E] = cvec.astype(ml_dtypes.bfloat16)
    cfull[:, E:] = (ck / nck).astype(ml_dtypes.bfloat16)
    tq = tvec @ At_w.T + At_b                        # [V, 40]
    ntq = np.maximum(np.linalg.norm(tq, axis=1, keepdims=True), 1e-6)
    tfull = np.ones((V, TW), dtype=ml_dtypes.bfloat16)
    tfull[:, :DA] = (tq / ntq).astype(ml_dtypes.bfloat16)
    W2 = R_w @ Bc_w
    w2t = np.ascontiguousarray(W2.T).astype(ml_dtypes.bfloat16)
    b2 = (R_w @ Bc_b + R_b).astype(np.float32)
    b2f = np.ascontiguousarray(np.broadcast_to(b2, (128, E)))
    identd = np.eye(128, dtype=np.float32).astype(ml_dtypes.bfloat16)

    in_maps = []
    for c in range(NCORES):
        b0 = c * BL
        cit = batch_citems[b0:b0 + BL].ravel()       # [12800] b-major
        tit = batch_titems[b0:b0 + BL].ravel()       # [4096]
        cidx = np.ascontiguousarray(cit.reshape(NTC, 128).T.astype(np.int32))
        tidx = np.ascontiguousarray(tit.reshape(NTT, 128).T.astype(np.int32))
        sel = (pad_rows >= b0) & (pad_rows < b0 + BL)
        negm = np.zeros((1, NTOK), dtype=ml_dtypes.bfloat16)
        flat = (pad_rows[sel] - b0) * M + pad_cols[sel]
        negm[0, flat] = NEG
        in_maps.append({
            "cfull": cfull, "tfull": tfull,
            "cidx": cidx, "tidx": tidx, "negm": negm,
            "w2t": w2t, "b2f": b2f, "identd": identd,
        })

    if _nc_cache[0] is None:
        _nc_cache[0] = _build_bass()
    nc = _nc_cache[0]

    res = run_bass_kernel_spmd(nc, in_maps, list(range(NCORES)),
                               trace=_trace[0])
    _last_exec_ns[0] = res.exec_time_ns
    z = np.stack([r["zout"].astype(np.float32).reshape(BL, J, E)
                  for r in res.results], axis=0)
    return z.reshape(B, J, E)
